# revision 1
# baseline (speedup 1.0000x reference)
"""AffinityCosineLoss on 8 Trainium2 NeuronCores.

Math: with zn = l2norm(y_pred[:, :192]), latent = (zn@zn.T + 1)/2,
target[i,j] = 0.2 (both bg) / 0.01 (one bg) / lookup[y_i,y_j] (both valid),
loss = sum_{i<j} |latent - target| / (B*(B-1)/2).

The entire pairwise computation is fused into a single K=323 matmul
P @ Q.T = latent - target by concatenating feature blocks along K:
  c0: zn_i[0:128] / sqrt(2)                    (both sides)
  c1: -(lookup @ onehot(y_i)) on P, onehot(y_j) on Q
  c2: zn_i[128:192] / sqrt(2) (both sides), const 1/sqrt(2) (masked on Q
      pads), and two background rows: (b_i, 1) on P vs
      (-0.01 - 0.18*b_j, -0.01*b_j) on Q        (b = is_background)
Then loss_sum = sum |P@Q.T| over the computed blocks.

Sharding (triangle/cyclic): the 4096x4096 pair matrix is an 8x8 grid of
512x512 super-blocks. Core r computes blocks (r, (r+d) mod 8) for d=0..4;
the d=4 slot is zero-padded on cores 4..7 (each unordered off-diagonal
block pair appears exactly once; diagonal blocks once). Host combines:
total = 2*offdiag + 1*diag - diag_elements, /2, /npairs.
"""

import functools

import ml_dtypes
import numpy as np

B = 4096
D = 256
L = 128
D_USE = 192  # int(D * 0.75)
NB = 8  # super-block grid (512 rows each)
BLK = B // NB  # 512
NSLOT = 5  # col slots per core (d = 0..4)
NCOL = NSLOT * BLK  # 2560
NT = NCOL // 128  # 20 row-tiles of the Q-side input
NSTRIP = BLK // 128  # 4 lhsT strips
NG = 5  # pipeline groups
GT = NT // NG  # 4 tiles per group
N_CORES = 8
NORM_EPS = 1e-8
INV_SQRT2 = 0.7071067811865476

BF16 = ml_dtypes.bfloat16


def _enable_ldw_opt():
    """Flip walrus --enable-ldw-opt to true (dedupes back-to-back LDWEIGHTS
    with identical stationary operands; our main loop is ordered for it)."""
    import concourse.bass_utils as bu

    if getattr(bu, "_ldw_opt_patched", False):
        return
    orig = bu.run_command

    def run_command_ldw(argv, **kwargs):
        argv = [
            a.replace("--enable-ldw-opt=false", "--enable-ldw-opt=true")
            if isinstance(a, str)
            else a
            for a in argv
        ]
        return orig(argv, **kwargs)

    bu.run_command = run_command_ldw
    bu._ldw_opt_patched = True


def _build_bass():
    import concourse.bacc as bacc
    import concourse.mybir as mybir
    import concourse.tile as tile

    fp32 = mybir.dt.float32
    bf16 = mybir.dt.bfloat16
    i32 = mybir.dt.int32

    nc = bacc.Bacc("TRN2", debug=False, num_devices=N_CORES)

    ypq_d = nc.dram_tensor("ypq", [128, NT * D_USE], bf16, kind="ExternalInput")
    ylab_d = nc.dram_tensor("ylab", [128, NCOL], bf16, kind="ExternalInput")
    lkn_d = nc.dram_tensor("lkn", [L, L], bf16, kind="ExternalInput")
    qrows_d = nc.dram_tensor("qrows", [3, NCOL], bf16, kind="ExternalInput")
    pbg1_d = nc.dram_tensor("pbg1", [2, BLK], bf16, kind="ExternalInput")
    out_d = nc.dram_tensor("out", [128, NSTRIP * NSLOT], fp32, kind="ExternalOutput")

    AX = mybir.AxisListType
    ALU = mybir.AluOpType
    ACTF = mybir.ActivationFunctionType

    ypq_ap = ypq_d.ap().rearrange("p (t d) -> p t d", d=D_USE)

    with tile.TileContext(nc) as tc:
        with (
            tc.tile_pool(name="cst", bufs=1) as cst,
            tc.tile_pool(name="work", bufs=1) as work,
            tc.tile_pool(name="ps", bufs=1, space="PSUM") as pps,
        ):
            # ---- engine warmup (runs during the input DMA) ----
            # ACT: touch the activation tables so table loads happen during
            # the DMA.  PE: junk matmuls to flip the HAM clock gate early.
            wz = cst.tile([128, 128], bf16)
            nc.gpsimd.memset(wz[:], 0.0)
            wact = cst.tile([128, 1], fp32)
            nc.gpsimd.memset(wact[:], 1.0)
            wact2 = cst.tile([128, 1], fp32)
            nc.scalar.activation(wact2[:], wact[:], ACTF.Sqrt)
            nc.scalar.activation(wact2[:], wact[:], ACTF.Abs)
            nc.scalar.activation(wact2[:], wact[:], ACTF.Square)
            wp = pps.tile([128, BLK], fp32, tag="gv", bufs=1, name="wp")
            for wi in range(8):
                nc.tensor.matmul(
                    wp[:, 0:128], wz[:], wz[:], start=(wi == 0), stop=(wi == 7)
                )

            # ---- input DMAs ----
            # SP ring: ybc slot-0 (tiny, feeds gvp early), then ypf g0/g2/g4
            # and the small constant rows.  SWDGE ring: ypf g1/g3 + the rest
            # of the label broadcast.
            ypf_all = work.tile([128, NT, D_USE], bf16)
            ybc = work.tile([128, NCOL], bf16)
            pc2 = work.tile([67, BLK], bf16)
            nc.sync.dma_start(ypf_all[:, 0:GT, :], ypq_ap[:, 0:GT, :])
            nc.sync.dma_start(ybc[:, 0:BLK], ylab_d.ap()[:, 0:BLK])
            lkn = cst.tile([L, L], bf16)
            nc.sync.dma_start(lkn[:], lkn_d.ap())
            nc.sync.dma_start(pc2[64:65, :], qrows_d.ap()[0:1, 0:BLK])
            nc.sync.dma_start(pc2[65:67, :], pbg1_d.ap())
            for g in range(1, NG):
                gs = slice(g * GT, (g + 1) * GT)
                eng = nc.gpsimd if g in (1, 3) else nc.sync
                eng.dma_start(ypf_all[:, gs, :], ypq_ap[:, gs, :])
            nc.gpsimd.dma_start(ybc[:, BLK:], ylab_d.ap()[:, BLK:])

            # per-partition class index (for the one-hot compare)
            iota_c = cst.tile([128, 1], i32)
            nc.gpsimd.iota(iota_c[:], pattern=[[0, 1]], base=0, channel_multiplier=1)
            iota_f = cst.tile([128, 1], fp32)
            nc.vector.tensor_copy(iota_f[:], iota_c[:])

            # ---- identity for PE transposes ----
            iota_pf = cst.tile([128, 128], i32)
            nc.gpsimd.iota(iota_pf[:], pattern=[[-1, 128]], base=0, channel_multiplier=1)
            idn = cst.tile([128, 128], bf16)
            nc.gpsimd.tensor_scalar(idn[:], iota_pf[:], 0, None, op0=ALU.is_equal)

            # ---- chunk tiles ----
            qc0 = work.tile([128, NCOL], bf16)  # zn feats 0..127 (transposed)
            qc2 = work.tile([67, NCOL], bf16)  # feats 128..191 + const + 2 bg
            avt = work.tile([128, NCOL], bf16)  # one-hot labels (transposed)
            acc = work.tile([128, NSTRIP * NSLOT], fp32)
            acc2 = acc.rearrange("p (mp h s) -> p mp h s", mp=2, h=2)
            gvp = work.tile([128, BLK], bf16)

            # slot-0 one-hot + gvp as early as possible (PE is idle here)
            nc.vector.tensor_scalar(
                avt[:, 0:BLK], ybc[:, 0:BLK], iota_f[:, 0:1], None, op0=ALU.is_equal
            )
            gps = pps.tile([128, BLK], fp32, tag="gv", bufs=1, name="gps")
            nc.tensor.matmul(gps[:], lkn[:], avt[:, 0:BLK], start=True, stop=True)
            nc.vector.tensor_copy(gvp[:], gps[:])

            pending_drains = []

            def emit_dve_drains():
                for mp, g2, pstile in pending_drains:
                    if mp == 0:
                        # acc layout: [128, mp, 2, NSLOT] flattened -> u = m*NSLOT+g
                        nc.vector.tensor_reduce(
                            acc2[:, mp, :, g2],
                            pstile[:],
                            axis=AX.X,
                            op=ALU.add,
                            apply_absolute_value=True,
                        )

            def emit_act_drains():
                for mp, g2, pstile in pending_drains:
                    if mp == 1:
                        scr = work.tile([128, 2, BLK], bf16, tag="scr", bufs=2)
                        nc.scalar.activation(
                            scr[:], pstile[:], ACTF.Abs, accum_out=None
                        ) if False else nc.vector.tensor_reduce(
                            acc2[:, mp, :, g2],
                            pstile[:],
                            axis=AX.X,
                            op=ALU.add,
                            apply_absolute_value=True,
                        )
                pending_drains.clear()

            for g in range(NG):
                ypf = ypf_all[:, g * GT : (g + 1) * GT, :]

                # norms: 2 tiles on DVE (mult+reduce), 2 on ACT (square+accum)
                nrm = work.tile([128, GT], fp32, tag="nrm", bufs=2)
                for i in range(GT):
                    if i >= 2:
                        sqs = work.tile([128, D_USE], bf16, tag="sqs", bufs=2)
                        nc.scalar.activation(
                            sqs[:],
                            ypf[:, i, :],
                            ACTF.Square,
                            accum_out=nrm[:, i : i + 1],
                        )
                    else:
                        sqg = work.tile([128, D_USE], bf16, tag="sqg", bufs=2)
                        nc.vector.tensor_mul(sqg[:], ypf[:, i, :], ypf[:, i, :])
                        nc.vector.tensor_reduce(
                            nrm[:, i : i + 1],
                            sqg[:],
                            axis=AX.X,
                            op=ALU.add,
                        )
                # previous group's drains fill the gap while ACT computes
                # sqrt and PE finishes that group's matmuls
                emit_dve_drains()
                rt = work.tile([128, GT], fp32, tag="rt", bufs=2)
                nc.scalar.activation(rt[:], nrm[:], ACTF.Sqrt, scale=2.0)
                emit_act_drains()
                rtm = work.tile([128, GT], fp32, tag="rtm", bufs=2)
                nc.vector.tensor_scalar_max(
                    rtm[:], rt[:], 1.4142135623730951 * NORM_EPS
                )
                s = work.tile([128, GT], fp32, tag="s", bufs=2)
                nc.vector.reciprocal(s[:], rtm[:])

                zn = work.tile([128, GT, D_USE], bf16, tag="zn", bufs=2)
                for i in range(GT):
                    nc.vector.tensor_scalar_mul(
                        zn[:, i, :], ypf[:, i, :], s[:, i : i + 1]
                    )

                # PE transposes into this group's columns of qc0/qc2
                cs = slice(g * BLK, (g + 1) * BLK)
                pt0 = pps.tile([128, GT * 128], bf16, tag="pt0", bufs=1)
                for i in range(GT):
                    nc.tensor.matmul(
                        pt0[:, i * 128 : (i + 1) * 128],
                        zn[:, i, 0:128],
                        idn[:],
                        is_transpose=True,
                        start=(i == 0),
                        stop=(i == GT - 1),
                    )
                nc.vector.tensor_copy(qc0[:, cs], pt0[:])
                pt2 = pps.tile([64, GT * 128], bf16, tag="pt2", bufs=1)
                for i in range(GT):
                    nc.tensor.matmul(
                        pt2[:, i * 128 : (i + 1) * 128],
                        zn[:, i, 128:D_USE],
                        idn[:],
                        is_transpose=True,
                        start=(i == 0),
                        stop=(i == GT - 1),
                    )
                nc.scalar.copy(qc2[0:64, cs], pt2[:])

                if g > 0:
                    # one-hot labels for this slot (slot 0 done up front)
                    nc.vector.tensor_scalar(
                        avt[:, cs], ybc[:, cs], iota_f[:, 0:1], None, op0=ALU.is_equal
                    )
                else:
                    # P-side c2 rows 0..63 straight from the transpose PSUM,
                    # in parallel with the qc2 evacuation
                    nc.vector.tensor_copy(pc2[0:64, :], pt2[:])
                    nc.sync.dma_start(qc2[64:67, :], qrows_d.ap())

                # main blocks for this slot; strip pairs share a 2-bank
                # PSUM tile so each drain covers two blocks
                for mp in range(NSTRIP // 2):
                    ps = pps.tile([128, 2, BLK], fp32, tag="mm", bufs=2, name=f"ps_{mp}_{g}")
                    for h in range(2):
                        m = mp * 2 + h
                        ms = slice(m * 128, (m + 1) * 128)
                        nc.tensor.matmul(
                            ps[:, h, :], qc0[:, ms], qc0[:, cs], start=True, stop=False
                        )
                        nc.tensor.matmul(
                            ps[:, h, :], gvp[:, ms], avt[:, cs], start=False, stop=False
                        )
                        nc.tensor.matmul(
                            ps[:, h, :], pc2[:, ms], qc2[:, cs], start=False, stop=True
                        )
                    pending_drains.append((mp, g, ps))

            emit_dve_drains()
            emit_act_drains()
            nc.sync.dma_start(out_d.ap(), acc[:])

    nc.compile()
    return nc


@functools.lru_cache(maxsize=1)
def _get_nc():
    return _build_bass()


def _host_inputs(y_true, y_pred, lookup):
    """Build the 8 per-core input maps."""
    yt = np.asarray(y_true).astype(np.int64)
    yp = np.asarray(y_pred).astype(np.float32)
    lk = np.asarray(lookup).astype(np.float32)

    labf = yt.astype(np.float32)  # -1 .. 127
    bg = (yt == -1).astype(np.float32)

    lkn = (-lk).astype(BF16)

    in_maps = []
    weights = np.zeros((N_CORES, NSLOT), np.float64)
    for r in range(N_CORES):
        ypq = np.zeros((NCOL, D_USE), np.float32)  # assembled, then rearranged
        ylab = np.full((NCOL,), -2.0, np.float32)
        wrow = np.zeros((NCOL,), np.float32)
        qb1 = np.zeros((NCOL,), np.float32)
        qb2 = np.zeros((NCOL,), np.float32)
        for d in range(NSLOT):
            valid = d < 4 or r < 4
            if not valid:
                continue
            cb = (r + d) % NB
            sl = slice(d * BLK, (d + 1) * BLK)
            gsl = slice(cb * BLK, (cb + 1) * BLK)
            ypq[sl] = yp[gsl, :D_USE]
            ylab[sl] = labf[gsl]
            wrow[sl] = INV_SQRT2
            b = bg[gsl]
            qb1[sl] = -0.01 - 0.18 * b
            qb2[sl] = -0.01 * b
            weights[r, d] = 1.0 if d == 0 else 2.0
        pb1 = np.stack([bg[r * BLK : (r + 1) * BLK], np.ones(BLK, np.float32)])
        in_maps.append(
            {
                "ypq": np.ascontiguousarray(
                    ypq.reshape(NT, 128, D_USE).transpose(1, 0, 2).reshape(128, NT * D_USE)
                ).astype(BF16),
                "ylab": np.ascontiguousarray(
                    np.broadcast_to(ylab.astype(BF16), (128, NCOL))
                ),
                "lkn": lkn,
                "qrows": np.stack([wrow, qb1, qb2]).astype(BF16),
                "pbg1": pb1.astype(BF16),
            }
        )
    return in_maps, weights


def _combine(outs, weights, y_true, lookup):
    """outs: list of 8 dicts with 'out' [128, NSTRIP*NSLOT]."""
    yt = np.asarray(y_true).astype(np.int64)
    lk = np.asarray(lookup).astype(np.float64)

    total = 0.0
    for r in range(N_CORES):
        o = outs[r]["out"].astype(np.float64).reshape(128, NSTRIP, NSLOT)
        per_slot = o.sum(axis=(0, 1))  # [NSLOT]
        total += float((per_slot * weights[r]).sum())

    # diagonal correction: latent_ii = 1, target_ii = 0.2 (bg) or lookup[y,y]
    bgm = yt == -1
    idx = np.clip(yt, 0, L - 1)
    tdiag = np.where(bgm, 0.2, lk[idx, idx])
    diag_sum = float(np.abs(1.0 - tdiag).sum())

    n_pairs = B * (B - 1) // 2
    return np.float32((total - diag_sum) / 2.0 / n_pairs)


def kernel(y_true, y_pred, lookup):
    from concourse.bass_utils import run_bass_kernel_spmd

    nc = _get_nc()
    in_maps, weights = _host_inputs(y_true, y_pred, lookup)
    res = run_bass_kernel_spmd(nc, in_maps, core_ids=list(range(N_CORES)))
    return _combine(res.results, weights, y_true, lookup)



# revision 3
# speedup vs baseline: 1.1094x; 1.1094x over previous
"""AffinityCosineLoss on 8 Trainium2 NeuronCores — fp8 DoubleRow streaming matmul.

Math: with zn = l2norm(y_pred[:, :192]), latent = (zn@zn.T + 1)/2,
target[i,j] = 0.2 (both bg) / 0.01 (one bg) / lookup[y_i,y_j] (both valid),
loss = sum_{i<j} |latent - target| / (B*(B-1)/2).

The entire pairwise map latent - target is a single K=323 contraction
P.T @ Q, fully packed on the HOST (fp32 math, then fp8 cast):
  rows   0:192  P = zn_i.T            Q = 0.5 * zn_j.T
  row    192    P = 1                 Q = 0.5            (the +1/2 of latent)
  row    193    P = b_i               Q = -0.01 - 0.18*b_j
  row    194    P = 1                 Q = -0.01*b_j      (b = is_background)
  rows 195:323  P = onehot(y_i)       Q = -lookup[:, y_j] * valid_j
The asymmetric 1.0 x 0.5 const split keeps the fp8 constants exact.

Sharding (triangle/cyclic): the 4096x4096 pair matrix is an 8x8 grid of
512x512 super-blocks. Core r computes blocks (r, (r+d) mod 8) for d=0..4;
the d=4 slot is zero-padded on cores 4..7. The x2 weight of off-diagonal
slots is baked into the Q columns (|2x| = 2|x|), so the device just
abs-sums everything. Host: total = sum - diag_correction, /2, /npairs.

Device work per core: 20 out-tiles [128,512]; each = two fp8 DoubleRow
matmuls (K-subtile pairs (128,128) and (34,34)) at 0.5 cycles/col,
ordered so consecutive matmuls share the stationary operand (LDW dedup).
PSUM: 5 x [128,2,512] units rotating through 8 banks (tag bufs=4).
Drains (abs + sum to one column of acc) alternate ACT / DVE.
"""

import functools

import ml_dtypes
import numpy as np

B = 4096
D = 256
L = 128
D_USE = 192  # int(D * 0.75)
NB = 8  # super-block grid (512 rows each)
BLK = B // NB  # 512
NSLOT = 5  # col slots per core (d = 0..4)
NCOL = NSLOT * BLK  # 2560
N_CORES = 8
NORM_EPS = 1e-8

KT = 324  # packed contraction rows: 323 + 1 zero pad
K2P = 34  # partition rows of the second DoubleRow chunk (2x34 = 68 >= 67)
NUNIT = 2 * NSLOT  # drain units: (slot, strip-half)

FP8 = ml_dtypes.float8_e4m3


def _enable_ldw_opt():
    """Flip walrus --enable-ldw-opt to true (dedupes back-to-back LDWEIGHTS
    with identical stationary operands; the main loop is ordered for it)."""
    import concourse.bass_utils as bu

    if getattr(bu, "_ldw_opt_patched", False):
        return
    orig = bu.run_command

    def run_command_ldw(argv, **kwargs):
        argv = [
            a.replace("--enable-ldw-opt=false", "--enable-ldw-opt=true")
            if isinstance(a, str)
            else a
            for a in argv
        ]
        return orig(argv, **kwargs)

    bu.run_command = run_command_ldw
    bu._ldw_opt_patched = True


def _build_bass():
    import concourse.bacc as bacc
    import concourse.mybir as mybir
    import concourse.tile as tile

    # NOTE: walrus --enable-ldw-opt rejects DoubleRow InstLdweights
    # ("not compatible with LDW optimization"), so it stays off here.

    fp32 = mybir.dt.float32
    bf16 = mybir.dt.bfloat16
    f8 = mybir.dt.float8e4

    nc = bacc.Bacc("TRN2", debug=False, num_devices=N_CORES)

    pt1_d = nc.dram_tensor("pt1", [128, 2 * BLK], f8, kind="ExternalInput")
    pt2_d = nc.dram_tensor("pt2", [K2P, 2 * BLK], f8, kind="ExternalInput")
    qt1_d = nc.dram_tensor("qt1", [128, NSLOT * 1024], f8, kind="ExternalInput")
    qt2_d = nc.dram_tensor("qt2", [K2P, NSLOT * 1024], f8, kind="ExternalInput")
    acc_d = nc.dram_tensor("acc", [128, NUNIT], fp32, kind="ExternalOutput")

    AX = mybir.AxisListType
    ALU = mybir.AluOpType
    ACTF = mybir.ActivationFunctionType
    DR = mybir.MatmulPerfMode.DoubleRow

    with tile.TileContext(nc) as tc:
        with (
            tc.tile_pool(name="cst", bufs=1) as cst,
            tc.tile_pool(name="work", bufs=1) as work,
            tc.tile_pool(name="ps", bufs=1, space="PSUM") as pps,
        ):
            # ---- SBUF tiles ----
            pt1s = work.tile([128, 2 * BLK], f8)
            pt2s = work.tile([K2P, 2 * BLK], f8)
            qt1s = work.tile([128, NSLOT * 1024], f8)
            qt2s = work.tile([K2P, NSLOT * 1024], f8)
            acc = work.tile([128, NUNIT], fp32)

            pt1v = pt1s.rearrange("p (t m) -> p t m", t=2)
            pt2v = pt2s.rearrange("p (t m) -> p t m", t=2)
            qt1v = qt1s.rearrange("p (g t n) -> p g t n", g=NSLOT, t=2)
            qt2v = qt2s.rearrange("p (g t n) -> p g t n", g=NSLOT, t=2)

            # ---- engine warmup ----
            # ACT: touch the Abs table during the input DMA.  PE: junk
            # matmuls to flip the HAM clock gate / start the pstate ramp.
            wz = cst.tile([128, 512], f8)
            nc.gpsimd.memset(wz[:], 0.0)
            wact = cst.tile([128, 1], fp32)
            nc.gpsimd.memset(wact[:], 1.0)
            wabs = cst.tile([128, 1], fp32)
            nc.scalar.activation(wabs[:], wact[:], ACTF.Abs)

            # ---- input DMAs (slot-sliced so wave 0 starts early) ----
            nc.sync.dma_start(pt1s[:], pt1_d.ap())
            nc.scalar.dma_start(pt2s[:], pt2_d.ap())
            for g in range(NSLOT):
                sl = slice(g * 1024, (g + 1) * 1024)
                eng = nc.sync if g in (0, 2, 4) else nc.scalar
                eng.dma_start(qt1s[:, sl], qt1_d.ap()[:, sl])
                nc.gpsimd.dma_start(qt2s[:, sl], qt2_d.ap()[:, sl])

            # PE warmup: standalone junk groups in the first psum unit's
            # banks (tag rotation makes real units wait on it — it's done
            # long before slot-0 data lands).
            wp = pps.tile([128, 2, BLK], fp32, tag="mm", bufs=4, name="wp")
            for wi in range(4):
                nc.tensor.matmul(
                    wp[:, wi % 2, :], wz[:, 0:128], wz[:], start=True, stop=True
                )

            # ---- main: 3 waves of slots {0,1}, {2,3}, {4} ----
            pending = []

            def drain(unit, u):
                if u % 2 == 0:
                    scr = work.tile([128, 2, BLK], bf16, tag="scr", bufs=2)
                    nc.scalar.activation(
                        scr[:], unit[:], ACTF.Abs, accum_out=acc[:, u : u + 1]
                    )
                else:
                    nc.vector.tensor_reduce(
                        acc[:, u : u + 1],
                        unit[:],
                        axis=AX.XY,
                        op=ALU.add,
                        apply_absolute_value=True,
                    )

            for wave in ((0, 1), (2, 3), (4,)):
                units = {}
                for g in wave:
                    for h in range(2):
                        units[(g, h)] = pps.tile(
                            [128, 2, BLK], fp32, tag="mm", bufs=4, name=f"u{g}_{h}"
                        )
                # drains of the previous wave overlap this wave's matmuls
                for unit, u in pending:
                    drain(unit, u)
                pending.clear()
                for m in range(4):
                    ms = slice(m * 128, (m + 1) * 128)
                    for chunk in range(2):
                        for g in wave:
                            lhsT = pt1v[:, :, ms] if chunk == 0 else pt2v[:, :, ms]
                            rhs = (
                                qt1v[:, g, :, :] if chunk == 0 else qt2v[:, g, :, :]
                            )
                            nc.tensor.matmul(
                                units[(g, m // 2)][:, m % 2, :],
                                lhsT,
                                rhs,
                                start=(chunk == 0),
                                stop=(chunk == 1),
                                perf_mode=DR,
                            )
                for g in wave:
                    for h in range(2):
                        pending.append((units[(g, h)], g * 2 + h))

            for unit, u in pending:
                drain(unit, u)
            nc.sync.dma_start(acc_d.ap(), acc[:])

    nc.compile()
    return nc


@functools.lru_cache(maxsize=1)
def _get_nc():
    return _build_bass()


def _pack_pq(y_true, y_pred, lookup):
    """Global [KT, B] P and Q fp32 matrices (see module docstring)."""
    yt = np.asarray(y_true).astype(np.int64)
    yp = np.asarray(y_pred).astype(np.float32)[:, :D_USE]
    lk = np.asarray(lookup).astype(np.float32)

    n = np.maximum(np.sqrt((yp * yp).sum(axis=1, keepdims=True)), NORM_EPS)
    zn = (yp / n).T  # [192, B]
    bg = (yt == -1).astype(np.float32)
    valid = (yt >= 0).astype(np.float32)
    idx = np.clip(yt, 0, L - 1)

    PG = np.zeros((KT, B), np.float32)
    QG = np.zeros((KT, B), np.float32)
    PG[0:D_USE] = zn
    QG[0:D_USE] = 0.5 * zn
    PG[192] = 1.0
    QG[192] = 0.5
    PG[193] = bg
    QG[193] = -0.01 - 0.18 * bg
    PG[194] = 1.0
    QG[194] = -0.01 * bg
    oh = np.zeros((L, B), np.float32)
    oh[idx, np.arange(B)] = valid
    PG[195 : 195 + L] = oh
    QG[195 : 195 + L] = -lk[:, idx] * valid[None, :]
    return PG, QG


def _host_inputs(y_true, y_pred, lookup):
    """Build the 8 per-core input maps."""
    PG, QG = _pack_pq(y_true, y_pred, lookup)

    def split_chunks(M):  # [KT, C] fp32 -> ([128,2,C], [K2P,2,C]) fp8
        c1 = M[0:256].reshape(2, 128, -1).transpose(1, 0, 2)
        c2 = M[256 : 256 + 2 * K2P].reshape(2, K2P, -1).transpose(1, 0, 2)
        return c1.astype(FP8), c2.astype(FP8)

    in_maps = []
    for r in range(N_CORES):
        qcore = np.zeros((KT, NCOL), np.float32)
        for d in range(NSLOT):
            if d == 4 and r >= 4:
                continue  # padded slot stays zero
            cb = (r + d) % NB
            w = 1.0 if d == 0 else 2.0
            qcore[:, d * BLK : (d + 1) * BLK] = (
                w * QG[:, cb * BLK : (cb + 1) * BLK]
            )
        pcore = PG[:, r * BLK : (r + 1) * BLK]
        p1, p2 = split_chunks(pcore)
        q1, q2 = split_chunks(qcore)
        # device layout: pt [p, t, m] flat; qt [p, g, t, n] flat
        q1 = q1.reshape(128, 2, NSLOT, BLK).transpose(0, 2, 1, 3)
        q2 = q2.reshape(K2P, 2, NSLOT, BLK).transpose(0, 2, 1, 3)
        in_maps.append(
            {
                "pt1": np.ascontiguousarray(p1.reshape(128, 2 * BLK)),
                "pt2": np.ascontiguousarray(p2.reshape(K2P, 2 * BLK)),
                "qt1": np.ascontiguousarray(q1.reshape(128, NSLOT * 1024)),
                "qt2": np.ascontiguousarray(q2.reshape(K2P, NSLOT * 1024)),
            }
        )
    return in_maps


def _combine(outs, y_true, lookup):
    """outs: list of 8 dicts with 'acc' [128, NUNIT]."""
    yt = np.asarray(y_true).astype(np.int64)
    lk = np.asarray(lookup).astype(np.float64)

    total = 0.0
    for r in range(N_CORES):
        total += float(outs[r]["acc"].astype(np.float64).sum())

    # diagonal correction: latent_ii = 1, target_ii = 0.2 (bg) or lookup[y,y]
    bgm = yt == -1
    idx = np.clip(yt, 0, L - 1)
    tdiag = np.where(bgm, 0.2, lk[idx, idx])
    diag_sum = float(np.abs(1.0 - tdiag).sum())

    n_pairs = B * (B - 1) // 2
    return np.float32((total - diag_sum) / 2.0 / n_pairs)


def kernel(y_true, y_pred, lookup):
    from concourse.bass_utils import run_bass_kernel_spmd

    nc = _get_nc()
    in_maps = _host_inputs(y_true, y_pred, lookup)
    res = run_bass_kernel_spmd(nc, in_maps, core_ids=list(range(N_CORES)))
    return _combine(res.results, y_true, lookup)


# revision 5
# speedup vs baseline: 1.2456x; 1.1228x over previous
"""AffinityCosineLoss on 8 Trainium2 NeuronCores — fp8 streaming matmul.

Math: with zn = l2norm(y_pred[:, :192]), latent = (zn@zn.T + 1)/2,
target[i,j] = 0.2 (both bg) / 0.01 (one bg) / lookup[y_i,y_j] (both valid),
loss = sum_{i<j} |latent - target| / (B*(B-1)/2).

The entire pairwise map latent - target is a single K=323 contraction
P.T @ Q, fully packed on the HOST (fp32 math, then fp8 cast):
  rows   0:192  P = zn_i.T            Q = 0.5 * zn_j.T
  row    192    P = 1                 Q = 0.5            (the +1/2 of latent)
  row    193    P = b_i               Q = -0.01 - 0.18*b_j
  row    194    P = 1                 Q = -0.01*b_j      (b = is_background)
  rows 195:323  P = onehot(y_i)       Q = -lookup[:, y_j] * valid_j
The asymmetric 1.0 x 0.5 const split keeps the fp8 constants exact.
K chunks: A = rows 0:128, B = rows 128:256, C = rows 256:323 (67).
Plain matmuls (no DoubleRow): full 128-col stationaries trigger the
compiler's Fast Weight Load, and --enable-ldw-opt dedupes the repeated
stationary across the slot-pair inner loop.

Sharding (triangle/cyclic): the 4096x4096 pair matrix is an 8x8 grid of
512x512 super-blocks. Core r computes blocks (r, (r+d) mod 8) for d=0..4;
the d=4 slot is zero-padded on cores 4..7. The x2 weight of off-diagonal
slots is baked into the Q columns (|2x| = 2|x|), so the device just
abs-sums everything. Host: total = sum - diag_correction, /2, /npairs.

Device: 20 out-tiles [128,512] in 3 waves of slots {0,1},{2,3},{4};
PSUM units [128,2,512] rotate through all 8 banks (tag bufs=4, first
rotation slot doubles as the PE-warmup target). Drains (abs + sum into
one acc column) alternate ACT (activation Abs accum_out) / DVE
(tensor_reduce) and overlap the next wave's matmuls.
"""

import functools

import ml_dtypes
import numpy as np

B = 4096
D = 256
L = 128
D_USE = 192  # int(D * 0.75)
NB = 8  # super-block grid (512 rows each)
BLK = B // NB  # 512
NSLOT = 5  # col slots per core (d = 0..4)
NCOL = NSLOT * BLK  # 2560
N_CORES = 8
NORM_EPS = 1e-8

KT = 323  # contraction rows
KC = 67  # rows of chunk C
NUNIT = 2 * NSLOT  # drain units: (slot, strip-half)

FP8 = ml_dtypes.float8_e4m3


def _enable_ldw_opt():
    """Flip walrus --enable-ldw-opt to true (dedupes back-to-back LDWEIGHTS
    with identical stationary operands; the main loop is ordered for it)."""
    import concourse.bass_utils as bu

    if getattr(bu, "_ldw_opt_patched", False):
        return
    orig = bu.run_command

    def run_command_ldw(argv, **kwargs):
        argv = [
            a.replace("--enable-ldw-opt=false", "--enable-ldw-opt=true")
            if isinstance(a, str)
            else a
            for a in argv
        ]
        return orig(argv, **kwargs)

    bu.run_command = run_command_ldw
    bu._ldw_opt_patched = True


def _build_bass():
    import concourse.bacc as bacc
    import concourse.mybir as mybir
    import concourse.tile as tile

    # NOTE: walrus --enable-ldw-opt rejects these fp8 InstLdweights
    # ("not compatible with LDW optimization"), so it stays off; the
    # PE's 64-deep reorder window still pulls LDWEIGHTS ahead.

    fp32 = mybir.dt.float32
    bf16 = mybir.dt.bfloat16
    f8 = mybir.dt.float8e4

    nc = bacc.Bacc("TRN2", debug=False, num_devices=N_CORES)

    pab_d = nc.dram_tensor("pab", [128, 2 * BLK], f8, kind="ExternalInput")
    pc_d = nc.dram_tensor("pc", [KC, BLK], f8, kind="ExternalInput")
    qab_d = nc.dram_tensor("qab", [128, NSLOT * 1024], f8, kind="ExternalInput")
    qc_d = nc.dram_tensor("qc", [KC, NSLOT * BLK], f8, kind="ExternalInput")
    acc_d = nc.dram_tensor("acc", [128, NUNIT], fp32, kind="ExternalOutput")

    AX = mybir.AxisListType
    ALU = mybir.AluOpType
    ACTF = mybir.ActivationFunctionType

    with tile.TileContext(nc) as tc:
        with (
            tc.tile_pool(name="cst", bufs=1) as cst,
            tc.tile_pool(name="work", bufs=1) as work,
            tc.tile_pool(name="ps", bufs=1, space="PSUM") as pps,
        ):
            # ---- SBUF tiles ----
            pabs = work.tile([128, 2 * BLK], f8)
            pcs = work.tile([KC, BLK], f8)
            qabs = work.tile([128, NSLOT * 1024], f8)
            qcs = work.tile([KC, NSLOT * BLK], f8)
            acc = work.tile([128, NUNIT], fp32)

            pabv = pabs.rearrange("p (c m) -> p c m", c=2)
            qabv = qabs.rearrange("p (g c n) -> p g c n", g=NSLOT, c=2)
            qcv = qcs.rearrange("p (g n) -> p g n", g=NSLOT)

            # ---- input DMAs first (wave-sliced), split over both HWDGE
            # queues; all issues precede any other engine work ----
            nc.sync.dma_start(pabs[:], pab_d.ap())
            nc.scalar.dma_start(pcs[:], pc_d.ap())
            for w, (eng_ab, eng_c) in enumerate(
                [(nc.sync, nc.scalar), (nc.sync, nc.scalar), (nc.sync, nc.scalar)]
            ):
                ab = slice(w * 2048, min((w + 1) * 2048, NSLOT * 1024))
                c = slice(w * 1024, min((w + 1) * 1024, NSLOT * BLK))
                eng_ab.dma_start(qabs[:, ab], qab_d.ap()[:, ab])
                eng_c.dma_start(qcs[:, c], qc_d.ap()[:, c])

            # ---- engine warmup ----
            # ACT touches the Abs table during the DMA; PE runs junk
            # matmuls to start the HAM ramp while slot-0 data lands.
            wz = cst.tile([128, 512], f8)
            nc.gpsimd.memset(wz[:], 0.0)
            wact = cst.tile([128, 1], fp32)
            nc.gpsimd.memset(wact[:], 1.0)
            wabs = cst.tile([128, 1], fp32)
            nc.scalar.activation(wabs[:], wact[:], ACTF.Abs)

            wp = pps.tile([128, 2, BLK], fp32, tag="mm", bufs=4, name="wp")
            for wi in range(6):
                nc.tensor.matmul(
                    wp[:, wi % 2, :], wz[:, 0:128], wz[:], start=True, stop=True
                )

            # ---- main: 3 waves of slots {0,1}, {2,3}, {4} ----
            pending = []

            def drain(unit, u):
                if u % 2 == 0:
                    scr = work.tile([128, 2, BLK], bf16, tag="scr", bufs=2)
                    nc.scalar.activation(
                        scr[:], unit[:], ACTF.Abs, accum_out=acc[:, u : u + 1]
                    )
                else:
                    nc.vector.tensor_reduce(
                        acc[:, u : u + 1],
                        unit[:],
                        axis=AX.XY,
                        op=ALU.add,
                        apply_absolute_value=True,
                    )

            for wave in ((0, 1), (2, 3), (4,)):
                units = {}
                for g in wave:
                    for h in range(2):
                        units[(g, h)] = pps.tile(
                            [128, 2, BLK], fp32, tag="mm", bufs=4, name=f"u{g}_{h}"
                        )
                # drains of the previous wave overlap this wave's matmuls
                for unit, u in pending:
                    drain(unit, u)
                pending.clear()
                for m in range(4):
                    ms = slice(m * 128, (m + 1) * 128)
                    for c in range(3):
                        for g in wave:
                            lhsT = pcs[:, ms] if c == 2 else pabv[:, c, ms]
                            rhs = qcv[:, g, :] if c == 2 else qabv[:, g, c, :]
                            nc.tensor.matmul(
                                units[(g, m // 2)][:, m % 2, :],
                                lhsT,
                                rhs,
                                start=(c == 0),
                                stop=(c == 2),
                            )
                for g in wave:
                    for h in range(2):
                        pending.append((units[(g, h)], g * 2 + h))

            for unit, u in pending:
                drain(unit, u)
            nc.sync.dma_start(acc_d.ap(), acc[:])

    nc.compile()
    return nc


@functools.lru_cache(maxsize=1)
def _get_nc():
    return _build_bass()


def _pack_pq(y_true, y_pred, lookup):
    """Global [KT, B] P and Q fp32 matrices (see module docstring)."""
    yt = np.asarray(y_true).astype(np.int64)
    yp = np.asarray(y_pred).astype(np.float32)[:, :D_USE]
    lk = np.asarray(lookup).astype(np.float32)

    n = np.maximum(np.sqrt((yp * yp).sum(axis=1, keepdims=True)), NORM_EPS)
    zn = (yp / n).T  # [192, B]
    bg = (yt == -1).astype(np.float32)
    valid = (yt >= 0).astype(np.float32)
    idx = np.clip(yt, 0, L - 1)

    PG = np.zeros((KT, B), np.float32)
    QG = np.zeros((KT, B), np.float32)
    PG[0:D_USE] = zn
    QG[0:D_USE] = 0.5 * zn
    PG[192] = 1.0
    QG[192] = 0.5
    PG[193] = bg
    QG[193] = -0.01 - 0.18 * bg
    PG[194] = 1.0
    QG[194] = -0.01 * bg
    oh = np.zeros((L, B), np.float32)
    oh[idx, np.arange(B)] = valid
    PG[195 : 195 + L] = oh
    QG[195 : 195 + L] = -lk[:, idx] * valid[None, :]
    return PG, QG


def _host_inputs(y_true, y_pred, lookup):
    """Build the 8 per-core input maps."""
    PG, QG = _pack_pq(y_true, y_pred, lookup)

    in_maps = []
    for r in range(N_CORES):
        qcore = np.zeros((KT, NCOL), np.float32)
        for d in range(NSLOT):
            if d == 4 and r >= 4:
                continue  # padded slot stays zero
            cb = (r + d) % NB
            w = 1.0 if d == 0 else 2.0
            qcore[:, d * BLK : (d + 1) * BLK] = (
                w * QG[:, cb * BLK : (cb + 1) * BLK]
            )
        pcore = PG[:, r * BLK : (r + 1) * BLK]
        p8 = pcore.astype(FP8)
        q8 = qcore.astype(FP8)
        # device layout: pab [p, c, m]; qab [p, g, c, n]; qc [p, g, n]
        pab = p8[0:256].reshape(2, 128, BLK).transpose(1, 0, 2)
        qab = (
            q8[0:256]
            .reshape(2, 128, NSLOT, BLK)
            .transpose(1, 2, 0, 3)
        )
        in_maps.append(
            {
                "pab": np.ascontiguousarray(pab.reshape(128, 2 * BLK)),
                "pc": np.ascontiguousarray(p8[256:KT]),
                "qab": np.ascontiguousarray(qab.reshape(128, NSLOT * 1024)),
                "qc": np.ascontiguousarray(q8[256:KT]),
            }
        )
    return in_maps


def _combine(outs, y_true, lookup):
    """outs: list of 8 dicts with 'acc' [128, NUNIT]."""
    yt = np.asarray(y_true).astype(np.int64)
    lk = np.asarray(lookup).astype(np.float64)

    total = 0.0
    for r in range(N_CORES):
        total += float(outs[r]["acc"].astype(np.float64).sum())

    # diagonal correction: latent_ii = 1, target_ii = 0.2 (bg) or lookup[y,y]
    bgm = yt == -1
    idx = np.clip(yt, 0, L - 1)
    tdiag = np.where(bgm, 0.2, lk[idx, idx])
    diag_sum = float(np.abs(1.0 - tdiag).sum())

    n_pairs = B * (B - 1) // 2
    return np.float32((total - diag_sum) / 2.0 / n_pairs)


def kernel(y_true, y_pred, lookup):
    from concourse.bass_utils import run_bass_kernel_spmd

    nc = _get_nc()
    in_maps = _host_inputs(y_true, y_pred, lookup)
    res = run_bass_kernel_spmd(nc, in_maps, core_ids=list(range(N_CORES)))
    return _combine(res.results, y_true, lookup)


# revision 7
# speedup vs baseline: 1.2552x; 1.0077x over previous
"""AffinityCosineLoss on 8 Trainium2 NeuronCores — fp8 streaming matmul.

Math: with zn = l2norm(y_pred[:, :192]), latent = (zn@zn.T + 1)/2,
target[i,j] = 0.2 (both bg) / 0.01 (one bg) / lookup[y_i,y_j] (both valid),
loss = sum_{i<j} |latent - target| / (B*(B-1)/2).

The entire pairwise map latent - target is a single K=323 contraction
P.T @ Q, fully packed on the HOST (fp32 math, then fp8 cast):
  rows   0:192  P = zn_i.T            Q = 0.5 * zn_j.T
  row    192    P = 1                 Q = 0.5            (the +1/2 of latent)
  row    193    P = b_i               Q = -0.01 - 0.18*b_j
  row    194    P = 1                 Q = -0.01*b_j      (b = is_background)
  rows 195:323  P = onehot(y_i)       Q = -lookup[:, y_j] * valid_j
The asymmetric 1.0 x 0.5 const split keeps the fp8 constants exact.
K chunks: A = rows 0:128, B = rows 128:256, C = rows 256:323 (67).
Plain matmuls (no DoubleRow): full 128-col stationaries trigger the
compiler's Fast Weight Load, and --enable-ldw-opt dedupes the repeated
stationary across the slot-pair inner loop.

Sharding (triangle/cyclic): the 4096x4096 pair matrix is an 8x8 grid of
512x512 super-blocks. Core r computes blocks (r, (r+d) mod 8) for d=0..4;
the d=4 slot is zero-padded on cores 4..7. The x2 weight of off-diagonal
slots is baked into the Q columns (|2x| = 2|x|), so the device just
abs-sums everything. Host: total = sum - diag_correction, /2, /npairs.

Device: 20 out-tiles [128,512] in 3 waves of slots {0,1},{2,3},{4};
PSUM units [128,2,512] rotate through all 8 banks (tag bufs=4, first
rotation slot doubles as the PE-warmup target). Drains (abs + sum into
one acc column) alternate ACT (activation Abs accum_out) / DVE
(tensor_reduce) and overlap the next wave's matmuls.
"""

import functools

import ml_dtypes
import numpy as np

B = 4096
D = 256
L = 128
D_USE = 192  # int(D * 0.75)
NB = 8  # super-block grid (512 rows each)
BLK = B // NB  # 512
NSLOT = 5  # col slots per core (d = 0..4)
NCOL = NSLOT * BLK  # 2560
N_CORES = 8
NORM_EPS = 1e-8

KT = 323  # contraction rows
KC = 67  # rows of chunk C
NUNIT = 2 * NSLOT  # drain units: (slot, strip-half)

FP8 = ml_dtypes.float8_e4m3


def _enable_ldw_opt():
    """Flip walrus --enable-ldw-opt to true (dedupes back-to-back LDWEIGHTS
    with identical stationary operands; the main loop is ordered for it)."""
    import concourse.bass_utils as bu

    if getattr(bu, "_ldw_opt_patched", False):
        return
    orig = bu.run_command

    def run_command_ldw(argv, **kwargs):
        argv = [
            a.replace("--enable-ldw-opt=false", "--enable-ldw-opt=true")
            if isinstance(a, str)
            else a
            for a in argv
        ]
        return orig(argv, **kwargs)

    bu.run_command = run_command_ldw
    bu._ldw_opt_patched = True


def _build_bass():
    import concourse.bacc as bacc
    import concourse.mybir as mybir
    import concourse.tile as tile

    # NOTE: walrus --enable-ldw-opt rejects these fp8 InstLdweights
    # ("not compatible with LDW optimization"), so it stays off; the
    # PE's 64-deep reorder window still pulls LDWEIGHTS ahead.

    fp32 = mybir.dt.float32
    bf16 = mybir.dt.bfloat16
    f8 = mybir.dt.float8e4

    nc = bacc.Bacc("TRN2", debug=False, num_devices=N_CORES)

    pab_d = nc.dram_tensor("pab", [128, 2 * BLK], f8, kind="ExternalInput")
    pc_d = nc.dram_tensor("pc", [KC, BLK], f8, kind="ExternalInput")
    qab_d = nc.dram_tensor("qab", [128, NSLOT * 1024], f8, kind="ExternalInput")
    qc_d = nc.dram_tensor("qc", [KC, NSLOT * BLK], f8, kind="ExternalInput")
    acc_d = nc.dram_tensor("acc", [128, NUNIT], fp32, kind="ExternalOutput")

    AX = mybir.AxisListType
    ALU = mybir.AluOpType
    ACTF = mybir.ActivationFunctionType

    with tile.TileContext(nc) as tc:
        with (
            tc.tile_pool(name="cst", bufs=1) as cst,
            tc.tile_pool(name="work", bufs=1) as work,
            tc.tile_pool(name="ps", bufs=1, space="PSUM") as pps,
        ):
            # ---- SBUF tiles ----
            pabs = work.tile([128, 2 * BLK], f8)
            pcs = work.tile([KC, BLK], f8)
            qabs = work.tile([128, NSLOT * 1024], f8)
            qcs = work.tile([KC, NSLOT * BLK], f8)
            acc = work.tile([128, NUNIT], fp32)

            pabv = pabs.rearrange("p (c m) -> p c m", c=2)
            qabv = qabs.rearrange("p (g c n) -> p g c n", g=NSLOT, c=2)
            qcv = qcs.rearrange("p (g n) -> p g n", g=NSLOT)

            # ---- input DMAs first, all on the sync HWDGE ring (the
            # scalar/Act ring measured ~10x slower), wave-sliced in
            # consumption order; scalar stays free for the ACT drains ----
            nc.sync.dma_start(pabs[:], pab_d.ap())
            nc.sync.dma_start(pcs[:], pc_d.ap())
            for w in range(3):
                ab = slice(w * 2048, min((w + 1) * 2048, NSLOT * 1024))
                c = slice(w * 1024, min((w + 1) * 1024, NSLOT * BLK))
                nc.sync.dma_start(qabs[:, ab], qab_d.ap()[:, ab])
                nc.sync.dma_start(qcs[:, c], qc_d.ap()[:, c])

            # ---- engine warmup ----
            # ACT touches the Abs table during the DMA; PE runs junk
            # matmuls to start the HAM ramp while slot-0 data lands.
            wz = cst.tile([128, 512], f8)
            nc.gpsimd.memset(wz[:], 0.0)
            wact = cst.tile([128, 1], fp32)
            nc.gpsimd.memset(wact[:], 1.0)
            wabs = cst.tile([128, 1], fp32)
            nc.scalar.activation(wabs[:], wact[:], ACTF.Abs)

            wp = pps.tile([128, 2, BLK], fp32, tag="mm", bufs=4, name="wp")
            for wi in range(4):
                nc.tensor.matmul(
                    wp[:, wi % 2, :], wz[:, 0:128], wz[:], start=True, stop=True
                )

            # ---- main: 3 waves of slots {0,1}, {2,3}, {4} ----
            pending = []

            def drain(unit, u):
                if u % 2 == 0:
                    scr = work.tile([128, 2, BLK], bf16, tag="scr", bufs=2)
                    nc.scalar.activation(
                        scr[:], unit[:], ACTF.Abs, accum_out=acc[:, u : u + 1]
                    )
                else:
                    nc.vector.tensor_reduce(
                        acc[:, u : u + 1],
                        unit[:],
                        axis=AX.XY,
                        op=ALU.add,
                        apply_absolute_value=True,
                    )

            for wave in ((0, 1), (2, 3), (4,)):
                units = {}
                for g in wave:
                    for h in range(2):
                        units[(g, h)] = pps.tile(
                            [128, 2, BLK], fp32, tag="mm", bufs=4, name=f"u{g}_{h}"
                        )
                # drains of the previous wave overlap this wave's matmuls
                for unit, u in pending:
                    drain(unit, u)
                pending.clear()
                for m in range(4):
                    ms = slice(m * 128, (m + 1) * 128)
                    for c in range(3):
                        for g in wave:
                            lhsT = pcs[:, ms] if c == 2 else pabv[:, c, ms]
                            rhs = qcv[:, g, :] if c == 2 else qabv[:, g, c, :]
                            nc.tensor.matmul(
                                units[(g, m // 2)][:, m % 2, :],
                                lhsT,
                                rhs,
                                start=(c == 0),
                                stop=(c == 2),
                            )
                for g in wave:
                    for h in range(2):
                        pending.append((units[(g, h)], g * 2 + h))

            for unit, u in pending:
                drain(unit, u)
            nc.sync.dma_start(acc_d.ap(), acc[:])

    nc.compile()
    return nc


@functools.lru_cache(maxsize=1)
def _get_nc():
    return _build_bass()


def _pack_pq(y_true, y_pred, lookup):
    """Global [KT, B] P and Q fp32 matrices (see module docstring)."""
    yt = np.asarray(y_true).astype(np.int64)
    yp = np.asarray(y_pred).astype(np.float32)[:, :D_USE]
    lk = np.asarray(lookup).astype(np.float32)

    n = np.maximum(np.sqrt((yp * yp).sum(axis=1, keepdims=True)), NORM_EPS)
    zn = (yp / n).T  # [192, B]
    bg = (yt == -1).astype(np.float32)
    valid = (yt >= 0).astype(np.float32)
    idx = np.clip(yt, 0, L - 1)

    PG = np.zeros((KT, B), np.float32)
    QG = np.zeros((KT, B), np.float32)
    PG[0:D_USE] = zn
    QG[0:D_USE] = 0.5 * zn
    PG[192] = 1.0
    QG[192] = 0.5
    PG[193] = bg
    QG[193] = -0.01 - 0.18 * bg
    PG[194] = 1.0
    QG[194] = -0.01 * bg
    oh = np.zeros((L, B), np.float32)
    oh[idx, np.arange(B)] = valid
    PG[195 : 195 + L] = oh
    QG[195 : 195 + L] = -lk[:, idx] * valid[None, :]
    return PG, QG


def _host_inputs(y_true, y_pred, lookup):
    """Build the 8 per-core input maps."""
    PG, QG = _pack_pq(y_true, y_pred, lookup)

    in_maps = []
    for r in range(N_CORES):
        qcore = np.zeros((KT, NCOL), np.float32)
        for d in range(NSLOT):
            if d == 4 and r >= 4:
                continue  # padded slot stays zero
            cb = (r + d) % NB
            w = 1.0 if d == 0 else 2.0
            qcore[:, d * BLK : (d + 1) * BLK] = (
                w * QG[:, cb * BLK : (cb + 1) * BLK]
            )
        pcore = PG[:, r * BLK : (r + 1) * BLK]
        p8 = pcore.astype(FP8)
        q8 = qcore.astype(FP8)
        # device layout: pab [p, c, m]; qab [p, g, c, n]; qc [p, g, n]
        pab = p8[0:256].reshape(2, 128, BLK).transpose(1, 0, 2)
        qab = (
            q8[0:256]
            .reshape(2, 128, NSLOT, BLK)
            .transpose(1, 2, 0, 3)
        )
        in_maps.append(
            {
                "pab": np.ascontiguousarray(pab.reshape(128, 2 * BLK)),
                "pc": np.ascontiguousarray(p8[256:KT]),
                "qab": np.ascontiguousarray(qab.reshape(128, NSLOT * 1024)),
                "qc": np.ascontiguousarray(q8[256:KT]),
            }
        )
    return in_maps


def _combine(outs, y_true, lookup):
    """outs: list of 8 dicts with 'acc' [128, NUNIT]."""
    yt = np.asarray(y_true).astype(np.int64)
    lk = np.asarray(lookup).astype(np.float64)

    total = 0.0
    for r in range(N_CORES):
        total += float(outs[r]["acc"].astype(np.float64).sum())

    # diagonal correction: latent_ii = 1, target_ii = 0.2 (bg) or lookup[y,y]
    bgm = yt == -1
    idx = np.clip(yt, 0, L - 1)
    tdiag = np.where(bgm, 0.2, lk[idx, idx])
    diag_sum = float(np.abs(1.0 - tdiag).sum())

    n_pairs = B * (B - 1) // 2
    return np.float32((total - diag_sum) / 2.0 / n_pairs)


def kernel(y_true, y_pred, lookup):
    from concourse.bass_utils import run_bass_kernel_spmd

    nc = _get_nc()
    in_maps = _host_inputs(y_true, y_pred, lookup)
    res = run_bass_kernel_spmd(nc, in_maps, core_ids=list(range(N_CORES)))
    return _combine(res.results, y_true, lookup)
